# revision 5
# baseline (speedup 1.0000x reference)
"""Trainium2 Bass kernel for a 2-layer LSTM + dense + softmax-CE loss.

Model (from the reference):
  B, T, V, E, H = 4096, 80, 80, 8, 256
  x  = emb[features]                  # [B, T, E]
  h1 = LSTM(x;  W1, b1)               # TF BasicLSTMCell, gates (i, j, f, o)
  h2 = LSTM(h1; W2, b2)
  pred = h2[:, -1] @ Wd + bd          # [B, V]
  loss = mean(softmax_xent(pred, labels))

Sharding: pure data parallelism — batch 4096 split 512/core across 8 cores,
weights replicated. Per-core device kernel computes the 512 per-row losses;
host averages the 4096 rows.

v2 design (ACT-engine-bound):
  - Hidden dim on partitions, batch (512) on the free dim; gate order
    permuted to [i, f, j, o] so one fused sigmoid covers (i, f) and the
    (j, o) pair splits tanh/sigmoid — 8 big ACT instructions per step
    instead of 20 small ones.
  - Recurrence matmuls in fp8-e4m3 with MatmulPerfMode.DoubleRow
    (K=256 per pass, 2x rate): L1 is one DR pass + the bf16 x-pass (K=9,
    with a ones row folding b1 + forget bias); L2 is two DR passes plus a
    K=1 ones-pass folding b2 + forget bias. No ACT bias reads anywhere.
  - PSUM as two [128, 2048] (4-bank) phase buffers; each layer-step is two
    phases (i,f) / (j,o) so PE fills one phase while ACT drains the other.
  - Software pipelining with L2 lagging L1 by one step: ACT alternates
    L1(t) and L2(t-1) blocks, each block's matmul inputs are ready a full
    slot ahead, so ACT (the bottleneck engine at ~10.9 us/step) never
    stalls on the recurrence chain.
  - DVE does the c/h elementwise chain fused over both 128-chunks
    (N=1024); h is written directly as the fp8 [128, 2, 512] DoubleRow
    rhs tile.
"""

from contextlib import ExitStack

import numpy as np

B, T, V, E, H = 4096, 80, 80, 8, 256
FORGET_BIAS = 1.0
NCORES = 8
BL = B // NCORES          # 512 batch rows per core
NB = BL // 128            # 4 batch tiles of 128 for the loss stage

_CACHE = {}


def _build_nc(fb_only: bool, T_steps=T):
    import concourse.tile as tile
    from concourse import bacc, mybir

    f32 = mybir.dt.float32
    bf16 = mybir.dt.bfloat16
    fp8 = mybir.dt.float8e4
    AF = mybir.ActivationFunctionType
    OP = mybir.AluOpType
    DR = mybir.MatmulPerfMode.DoubleRow

    nc = bacc.Bacc("TRN2", target_bir_lowering=False, debug=False)

    XT = nc.dram_tensor("XT", [T_steps, E + 1, BL], bf16, kind="ExternalInput")
    OH = nc.dram_tensor("OH", [BL, V], f32, kind="ExternalInput")
    W1X = nc.dram_tensor("W1X", [E + 1, 4 * H], bf16, kind="ExternalInput")
    W1H = nc.dram_tensor("W1H", [128, 2, 4 * H], fp8, kind="ExternalInput")
    W2A = nc.dram_tensor("W2A", [128, 2, 4 * H], fp8, kind="ExternalInput")
    W2B = nc.dram_tensor("W2B", [128, 2, 4 * H], fp8, kind="ExternalInput")
    B2R = nc.dram_tensor("B2R", [1, 4 * H], bf16, kind="ExternalInput")
    WD = nc.dram_tensor("WD", [H, V], bf16, kind="ExternalInput")
    BD = nc.dram_tensor("BD", [1, V], bf16, kind="ExternalInput")
    LOSS = nc.dram_tensor("LOSS", [NB, 128], f32, kind="ExternalOutput")

    # Gate order in PSUM / weight columns: [i, f, j, o], 2 M-tiles each.
    # Phase P1 = M-tiles 0..3 (i, f): one fused sigmoid N=2048.
    # Phase P2 = M-tiles 4..7 (j, o): tanh N=1024 then sigmoid N=1024.
    # L2 bias ones-pass covers f tiles {2,3} when b2 == 0, else all 8.
    bias_tiles = {2, 3} if fb_only else set(range(8))

    with tile.TileContext(nc) as tc, ExitStack() as ctx:
        wp = ctx.enter_context(tc.tile_pool(name="weights", bufs=1))
        sp = ctx.enter_context(tc.tile_pool(name="state", bufs=1))
        gp = ctx.enter_context(tc.tile_pool(name="gates", bufs=2))
        hp = ctx.enter_context(tc.tile_pool(name="h", bufs=2))
        pp = ctx.enter_context(tc.tile_pool(name="psum", bufs=2, space="PSUM"))
        xp = ctx.enter_context(tc.tile_pool(name="xstream", bufs=4))
        lp = ctx.enter_context(tc.tile_pool(name="loss", bufs=1))

        # ---- static loads, ordered by first use ----
        xt0 = xp.tile([E + 1, BL], bf16, tag="xt", name="xt0")
        nc.sync.dma_start(xt0[:], XT[0])
        w1x = wp.tile([E + 1, 4 * H], bf16, tag="w1x")
        nc.sync.dma_start(w1x[:], W1X[:])
        w1h = wp.tile([128, 2, 4 * H], fp8, tag="w1h")
        nc.sync.dma_start(w1h[:], W1H[:])
        w2a = wp.tile([128, 2, 4 * H], fp8, tag="w2a")
        nc.sync.dma_start(w2a[:], W2A[:])
        b2r = wp.tile([1, 4 * H], bf16, tag="b2r")
        nc.sync.dma_start(b2r[:], B2R[:])
        w2b = wp.tile([128, 2, 4 * H], fp8, tag="w2b")
        nc.sync.dma_start(w2b[:], W2B[:])
        ones_f = wp.tile([1, BL], f32, tag="ones_f")
        nc.vector.memset(ones_f[:], 1.0)
        ones = wp.tile([1, BL], bf16, tag="ones")
        nc.vector.tensor_copy(ones[:], ones_f[:])
        wd = []
        for s in range(2):
            t_ = wp.tile([128, V], bf16, tag=f"wd{s}")
            nc.sync.dma_start(t_[:], WD[128 * s : 128 * (s + 1), :])
            wd.append(t_)
        bdt = wp.tile([1, V], bf16, tag="bdt")
        nc.sync.dma_start(bdt[:], BD[:])
        oh_tiles = []
        for m in range(NB):
            t_ = lp.tile([128, V], f32, tag=f"oh{m}", name=f"oh{m}")
            nc.sync.dma_start(t_[:], OH[128 * m : 128 * (m + 1), :])
            oh_tiles.append(t_)

        # persistent cell states, [128, 1024] f32: chunk s at cols s*512
        c1 = sp.tile([128, 2 * BL], f32, tag="c1", name="c1")
        c2 = sp.tile([128, 2 * BL], f32, tag="c2", name="c2")

        state = {"h2": None, "h2bf": None, "xt": None}

        def phase_mm(ps, mset, parts):
            """Emit matmuls for one 4-bank phase.

            ps: [128, 2048] psum tile; mset: global M-tile indices (len 4);
            parts: list of (lhsT_fn, rhs, perf_mode, on_tiles) in order.
            """
            for k, m in enumerate(mset):
                out = ps[:, 512 * k : 512 * (k + 1)]
                acts = [p for p in parts if m in p[3]]
                n = len(acts)
                for a, (lf, rhs, pm, _) in enumerate(acts):
                    nc.tensor.matmul(
                        out, lf(m), rhs, start=(a == 0), stop=(a == n - 1),
                        perf_mode=pm,
                    )

        def emit_layer(layer, t, h1_in):
            """One layer-step: 2 matmul phases, 4 ACT insts, DVE chain.

            h1_in: for layer 1 this is h1(t-1) (DR recurrence rhs); for
            layer 2 it is h1(t) (the forward input).
            """
            if layer == 1:
                xt = xt0 if t == 0 else state["xt"]
                parts = []
                if t > 0:
                    parts.append(
                        (lambda m: w1h[:, :, 128 * m : 128 * (m + 1)],
                         h1_in[:, :, :], DR, set(range(8))))
                parts.append(
                    (lambda m: w1x[:, 128 * m : 128 * (m + 1)],
                     xt[:], None, set(range(8))))
                c = c1
            else:
                parts = [
                    (lambda m: w2a[:, :, 128 * m : 128 * (m + 1)],
                     h1_in[:, :, :], DR, set(range(8)))]
                if t > 0:
                    h2p = state["h2"]
                    parts.append(
                        (lambda m: w2b[:, :, 128 * m : 128 * (m + 1)],
                         h2p[:, :, :], DR, set(range(8))))
                parts.append(
                    (lambda m: b2r[:, 128 * m : 128 * (m + 1)],
                     ones[:], None, bias_tiles))
                c = c2

            lname = f"L{layer}"
            ps1 = pp.tile([128, 2048], f32, tag="ph", name=f"{lname}p1_{t}")
            phase_mm(ps1, range(0, 4), parts)
            # ACT: fused sigmoid over (i, f)
            sif = gp.tile([128, 2048], bf16, tag=f"{lname}sif")
            nc.scalar.activation(sif[:], ps1[:], AF.Sigmoid)
            ps2 = pp.tile([128, 2048], f32, tag="ph", name=f"{lname}p2_{t}")
            phase_mm(ps2, range(4, 8), parts)
            tj = gp.tile([128, 1024], bf16, tag=f"{lname}tj")
            nc.scalar.activation(tj[:], ps2[:, 0:1024], AF.Tanh)
            so = gp.tile([128, 1024], bf16, tag=f"{lname}so")
            nc.scalar.activation(so[:], ps2[:, 1024:2048], AF.Sigmoid)

            # DVE: c update (c = c*sf + si*tj), overlapped with ACT above
            si = sif[:, 0:1024]
            sf = sif[:, 1024:2048]
            if t == 0:
                nc.vector.tensor_tensor(c[:], si, tj[:], op=OP.mult)
            else:
                nc.vector.tensor_tensor(c[:], c[:], sf, op=OP.mult)
                tmp = gp.tile([128, 1024], bf16, tag=f"{lname}tmp")
                nc.vector.tensor_tensor(tmp[:], si, tj[:], op=OP.mult)
                nc.vector.tensor_tensor(c[:], c[:], tmp[:], op=OP.add)
            th = gp.tile([128, 1024], bf16, tag=f"{lname}th")
            nc.scalar.activation(th[:], c[:], AF.Tanh)
            last2 = layer == 2 and t == T_steps - 1
            if not last2:
                hdr = hp.tile([128, 2, BL], fp8, tag=f"{lname}h")
                nc.vector.tensor_tensor(hdr[:, :, :], th[:], so[:], op=OP.mult)
                if layer == 2:
                    state["h2"] = hdr
                return hdr
            else:
                h2bf = hp.tile([128, 2 * BL], bf16, tag="h2bf")
                nc.vector.tensor_tensor(h2bf[:], th[:], so[:], op=OP.mult)
                state["h2bf"] = h2bf
                return None

        # ---- pipelined main loop: slot s runs L1(s) and L2(s-1) ----
        h1_cur = None  # h1(s) after the L1 half of slot s
        for s in range(T_steps + 1):
            h1_prev = h1_cur  # h1(s-1)
            if s < T_steps:
                if s + 1 < T_steps:
                    nxt = xp.tile([E + 1, BL], bf16, tag="xt", name=f"xt{s+1}")
                    nc.sync.dma_start(nxt[:], XT[s + 1])
                h1_cur = emit_layer(1, s, h1_prev)
                if s + 1 < T_steps:
                    state["xt"] = nxt
            if s >= 1:
                # L2(s-1) consumes h1(s-1)
                emit_layer(2, s - 1, h1_prev if s < T_steps else h1_cur)

        # ---- dense + softmax cross-entropy on the last h2 ----
        h2bf = state["h2bf"]
        pdall = pp.tile([128, 2048], f32, tag="ph", name="pdall")
        pds, nmxs, ses, pkss = [], [], [], []
        for m in range(NB):
            pd = pdall[:, 512 * m : 512 * m + V]
            ms = slice(128 * m, 128 * (m + 1))
            nc.tensor.matmul(pd, h2bf[:, ms], wd[0][:], start=True, stop=False)
            nc.tensor.matmul(pd, h2bf[:, BL + 128 * m : BL + 128 * (m + 1)],
                             wd[1][:], start=False, stop=False)
            nc.tensor.matmul(pd, ones[:, ms], bdt[:], start=False, stop=True)
            pds.append(pd)
            mx = lp.tile([128, 1], f32, tag=f"mx{m}")
            nc.vector.reduce_max(out=mx[:], in_=pd, axis=mybir.AxisListType.X)
            nmx = lp.tile([128, 1], f32, tag=f"nmx{m}")
            nc.vector.tensor_scalar_mul(nmx[:], mx[:], -1.0)
            nmxs.append(nmx)
        for m in range(NB):
            ex = lp.tile([128, V], f32, tag=f"ex{m}")
            se = lp.tile([128, 1], f32, tag=f"se{m}")
            nc.scalar.activation(ex[:], pds[m], AF.Exp, bias=nmxs[m][:],
                                 accum_out=se[:])
            ses.append(se)
        lses = []
        for m in range(NB):
            lse = lp.tile([128, 1], f32, tag=f"lse{m}")
            nc.scalar.activation(lse[:], ses[m][:], AF.Ln)
            lses.append(lse)
            pk = lp.tile([128, V], f32, tag=f"pk{m}")
            nc.vector.tensor_tensor(pk[:], pds[m], oh_tiles[m][:], op=OP.mult)
            pks = lp.tile([128, 1], f32, tag=f"pks{m}")
            nc.vector.reduce_sum(out=pks[:], in_=pk[:], axis=mybir.AxisListType.X)
            pkss.append(pks)
        for m in range(NB):
            # loss = max + lse - picked  (lse is ln(sum exp(pred - max)))
            l0 = lp.tile([128, 1], f32, tag=f"l0{m}")
            nc.vector.tensor_tensor(l0[:], lses[m][:], pkss[m][:], op=OP.subtract)
            l1_ = lp.tile([128, 1], f32, tag=f"l1{m}")
            nc.vector.tensor_tensor(l1_[:], l0[:], nmxs[m][:], op=OP.subtract)
            nc.sync.dma_start(LOSS[m, :], l1_[:, 0:1])

    nc.compile()
    return nc


# permutation (i, j, f, o) -> (i, f, j, o) on the 4H gate axis
def _perm():
    idx = np.arange(4 * H).reshape(4, H)
    return np.concatenate([idx[0], idx[2], idx[1], idx[3]])


def _dr(w):
    """[256, M] -> DoubleRow layout [128, 2, M]."""
    return np.ascontiguousarray(w.reshape(2, 128, -1).transpose(1, 0, 2))


def _prep_inputs(features, labels, emb, W1, b1, W2, b2, Wd, bd):
    """Host-side shard + layout prep. Returns (in_maps, fb_only)."""
    import ml_dtypes

    bf16 = ml_dtypes.bfloat16
    fp8 = ml_dtypes.float8_e4m3
    perm = _perm()

    features = np.asarray(features)
    labels = np.asarray(labels)
    emb = np.asarray(emb, dtype=np.float32)
    W1p = np.asarray(W1, dtype=np.float32)[:, perm]
    W2p = np.asarray(W2, dtype=np.float32)[:, perm]
    b1p = np.asarray(b1, dtype=np.float32)[perm].copy()
    b2p = np.asarray(b2, dtype=np.float32)[perm].copy()
    fb_only = not np.any(b2p)
    # forget gate now lives at columns H..2H
    b1p[H : 2 * H] += FORGET_BIAS
    b2p[H : 2 * H] += FORGET_BIAS

    W1X = np.ascontiguousarray(
        np.concatenate([W1p[:E], b1p[None, :]], axis=0).astype(bf16))
    W1H = _dr(W1p[E:]).astype(fp8)
    W2A = _dr(W2p[:H]).astype(fp8)
    W2B = _dr(W2p[H:]).astype(fp8)
    B2R = np.ascontiguousarray(b2p[None, :].astype(bf16))
    WDc = np.ascontiguousarray(np.asarray(Wd, dtype=np.float32).astype(bf16))
    BDc = np.ascontiguousarray(
        np.asarray(bd, dtype=np.float32).reshape(1, V).astype(bf16))

    x = emb[features]  # [B, T, E] f32
    eye = np.eye(V, dtype=np.float32)

    in_maps = []
    for c in range(NCORES):
        sl = slice(c * BL, (c + 1) * BL)
        xc = x[sl].transpose(1, 2, 0)  # [T, E, BL]
        xo = np.concatenate(
            [xc, np.ones((T, 1, BL), np.float32)], axis=1)  # [T, E+1, BL]
        oh = eye[labels[sl]]
        in_maps.append({
            "XT": np.ascontiguousarray(xo.astype(bf16)),
            "OH": np.ascontiguousarray(oh),
            "W1X": W1X, "W1H": W1H, "W2A": W2A, "W2B": W2B, "B2R": B2R,
            "WD": WDc, "BD": BDc,
        })
    return in_maps, fb_only


def _get_nc(fb_only):
    key = ("nc", fb_only)
    if key not in _CACHE:
        _CACHE[key] = _build_nc(fb_only)
    return _CACHE[key]


def _run(inputs, trace=False, **spmd_kwargs):
    from concourse.bass_utils import run_bass_kernel_spmd

    in_maps, fb_only = _prep_inputs(**inputs)
    nc = _get_nc(fb_only)
    res = run_bass_kernel_spmd(
        nc, in_maps, list(range(NCORES)), trace=trace, **spmd_kwargs
    )
    rows = np.concatenate(
        [np.asarray(r["LOSS"], np.float64).ravel() for r in res.results])
    loss = np.asarray(rows.mean(), dtype=np.float32)
    return loss, res


def kernel(**inputs):
    loss, _ = _run(inputs, trace=False)
    return loss


# revision 6
# speedup vs baseline: 1.2012x; 1.2012x over previous
"""Trainium2 Bass kernel for a 2-layer LSTM + dense + softmax-CE loss.

Model (from the reference):
  B, T, V, E, H = 4096, 80, 80, 8, 256
  x  = emb[features]                  # [B, T, E]
  h1 = LSTM(x;  W1, b1)               # TF BasicLSTMCell, gates (i, j, f, o)
  h2 = LSTM(h1; W2, b2)
  pred = h2[:, -1] @ Wd + bd          # [B, V]
  loss = mean(softmax_xent(pred, labels))

Sharding: pure data parallelism — batch 4096 split 512/core across 8 cores,
weights replicated. Per-core device kernel computes the 512 per-row losses;
host averages the 4096 rows.

v2 design (ACT-engine-bound):
  - Hidden dim on partitions, batch (512) on the free dim; gate order
    permuted to [i, f, j, o] so one fused sigmoid covers (i, f) and the
    (j, o) pair splits tanh/sigmoid — 8 big ACT instructions per step
    instead of 20 small ones.
  - Recurrence matmuls in fp8-e4m3 with MatmulPerfMode.DoubleRow
    (K=256 per pass, 2x rate): L1 is one DR pass + the bf16 x-pass (K=9,
    with a ones row folding b1 + forget bias); L2 is two DR passes plus a
    K=1 ones-pass folding b2 + forget bias. No ACT bias reads anywhere.
  - PSUM as two [128, 2048] (4-bank) phase buffers; each layer-step is two
    phases (i,f) / (j,o) so PE fills one phase while ACT drains the other.
  - Software pipelining with L2 lagging L1 by one step: ACT alternates
    L1(t) and L2(t-1) blocks, each block's matmul inputs are ready a full
    slot ahead, so ACT (the bottleneck engine at ~10.9 us/step) never
    stalls on the recurrence chain.
  - DVE does the c/h elementwise chain fused over both 128-chunks
    (N=1024); h is written directly as the fp8 [128, 2, 512] DoubleRow
    rhs tile.
"""

from contextlib import ExitStack

import numpy as np

B, T, V, E, H = 4096, 80, 80, 8, 256
FORGET_BIAS = 1.0
NCORES = 8
BL = B // NCORES          # 512 batch rows per core
NB = BL // 128            # 4 batch tiles of 128 for the loss stage

_CACHE = {}


def _build_nc(fb_only: bool, T_steps=T):
    import concourse.tile as tile
    from concourse import bacc, mybir

    f32 = mybir.dt.float32
    bf16 = mybir.dt.bfloat16
    fp8 = mybir.dt.float8e4
    AF = mybir.ActivationFunctionType
    OP = mybir.AluOpType
    DR = mybir.MatmulPerfMode.DoubleRow

    nc = bacc.Bacc("TRN2", target_bir_lowering=False, debug=False)

    XT = nc.dram_tensor("XT", [T_steps, E + 1, BL], bf16, kind="ExternalInput")
    OH = nc.dram_tensor("OH", [BL, V], f32, kind="ExternalInput")
    W1X = nc.dram_tensor("W1X", [E + 1, 4 * H], bf16, kind="ExternalInput")
    W1H = nc.dram_tensor("W1H", [128, 2, 4 * H], fp8, kind="ExternalInput")
    W2A = nc.dram_tensor("W2A", [128, 2, 4 * H], fp8, kind="ExternalInput")
    W2B = nc.dram_tensor("W2B", [128, 2, 4 * H], fp8, kind="ExternalInput")
    B2R = nc.dram_tensor("B2R", [1, 4 * H], bf16, kind="ExternalInput")
    WD = nc.dram_tensor("WD", [H, V], bf16, kind="ExternalInput")
    BD = nc.dram_tensor("BD", [1, V], bf16, kind="ExternalInput")
    LOSS = nc.dram_tensor("LOSS", [NB, 128], f32, kind="ExternalOutput")

    # Gate order in PSUM / weight columns: [i, f, j, o], 2 M-tiles each.
    # Phase P1 = M-tiles 0..3 (i, f): one fused sigmoid N=2048.
    # Phase P2 = M-tiles 4..7 (j, o): tanh N=1024 then sigmoid N=1024.
    # L2 bias ones-pass covers f tiles {2,3} when b2 == 0, else all 8.
    bias_tiles = {2, 3} if fb_only else set(range(8))

    with tile.TileContext(nc) as tc, ExitStack() as ctx:
        wp = ctx.enter_context(tc.tile_pool(name="weights", bufs=1))
        sp = ctx.enter_context(tc.tile_pool(name="state", bufs=1))
        gp = ctx.enter_context(tc.tile_pool(name="gates", bufs=2))
        hp = ctx.enter_context(tc.tile_pool(name="h", bufs=2))
        pp = ctx.enter_context(tc.tile_pool(name="psum", bufs=2, space="PSUM"))
        xp = ctx.enter_context(tc.tile_pool(name="xstream", bufs=4))
        lp = ctx.enter_context(tc.tile_pool(name="loss", bufs=1))

        # ---- static loads, ordered by first use ----
        xt0 = xp.tile([E + 1, BL], bf16, tag="xt", name="xt0")
        nc.sync.dma_start(xt0[:], XT[0])
        w1x = wp.tile([E + 1, 4 * H], bf16, tag="w1x")
        nc.sync.dma_start(w1x[:], W1X[:])
        w1h = wp.tile([128, 2, 4 * H], fp8, tag="w1h")
        nc.sync.dma_start(w1h[:], W1H[:])
        w2a = wp.tile([128, 2, 4 * H], fp8, tag="w2a")
        nc.sync.dma_start(w2a[:], W2A[:])
        b2r = wp.tile([1, 4 * H], bf16, tag="b2r")
        nc.sync.dma_start(b2r[:], B2R[:])
        w2b = wp.tile([128, 2, 4 * H], fp8, tag="w2b")
        nc.sync.dma_start(w2b[:], W2B[:])
        ones_f = wp.tile([1, BL], f32, tag="ones_f")
        nc.vector.memset(ones_f[:], 1.0)
        ones = wp.tile([1, BL], bf16, tag="ones")
        nc.vector.tensor_copy(ones[:], ones_f[:])
        wd = []
        for s in range(2):
            t_ = wp.tile([128, V], bf16, tag=f"wd{s}")
            nc.sync.dma_start(t_[:], WD[128 * s : 128 * (s + 1), :])
            wd.append(t_)
        bdt = wp.tile([1, V], bf16, tag="bdt")
        nc.sync.dma_start(bdt[:], BD[:])
        oh_tiles = []
        for m in range(NB):
            t_ = lp.tile([128, V], f32, tag=f"oh{m}", name=f"oh{m}")
            nc.sync.dma_start(t_[:], OH[128 * m : 128 * (m + 1), :])
            oh_tiles.append(t_)

        # persistent cell states, [128, 1024] bf16: chunk s at cols s*512
        # (bf16 keeps every DVE op in the 2x all-16-bit mode: 602 vs 1135 ns)
        c1 = sp.tile([128, 2 * BL], bf16, tag="c1", name="c1")
        c2 = sp.tile([128, 2 * BL], bf16, tag="c2", name="c2")

        state = {"h2": None, "h2bf": None, "xt": None}

        def phase_mm(ps, mset, parts):
            """Emit matmuls for one 4-bank phase.

            ps: [128, 2048] psum tile; mset: global M-tile indices (len 4);
            parts: list of (lhsT_fn, rhs, perf_mode, on_tiles) in order.
            """
            for k, m in enumerate(mset):
                out = ps[:, 512 * k : 512 * (k + 1)]
                acts = [p for p in parts if m in p[3]]
                n = len(acts)
                for a, (lf, rhs, pm, _) in enumerate(acts):
                    nc.tensor.matmul(
                        out, lf(m), rhs, start=(a == 0), stop=(a == n - 1),
                        perf_mode=pm,
                    )

        def emit_layer(layer, t, h1_in):
            """One layer-step: 2 matmul phases, 4 ACT insts, DVE chain.

            h1_in: for layer 1 this is h1(t-1) (DR recurrence rhs); for
            layer 2 it is h1(t) (the forward input).
            """
            if layer == 1:
                xt = xt0 if t == 0 else state["xt"]
                parts = []
                if t > 0:
                    parts.append(
                        (lambda m: w1h[:, :, 128 * m : 128 * (m + 1)],
                         h1_in[:, :, :], DR, set(range(8))))
                parts.append(
                    (lambda m: w1x[:, 128 * m : 128 * (m + 1)],
                     xt[:], None, set(range(8))))
                c = c1
            else:
                parts = [
                    (lambda m: w2a[:, :, 128 * m : 128 * (m + 1)],
                     h1_in[:, :, :], DR, set(range(8)))]
                if t > 0:
                    h2p = state["h2"]
                    parts.append(
                        (lambda m: w2b[:, :, 128 * m : 128 * (m + 1)],
                         h2p[:, :, :], DR, set(range(8))))
                parts.append(
                    (lambda m: b2r[:, 128 * m : 128 * (m + 1)],
                     ones[:], None, bias_tiles))
                c = c2

            lname = f"L{layer}"
            ps1 = pp.tile([128, 2048], f32, tag="ph", name=f"{lname}p1_{t}")
            phase_mm(ps1, range(0, 4), parts)
            # ACT: fused sigmoid over (i, f)
            sif = gp.tile([128, 2048], bf16, tag=f"{lname}sif")
            nc.scalar.activation(sif[:], ps1[:], AF.Sigmoid)
            ps2 = pp.tile([128, 2048], f32, tag="ph", name=f"{lname}p2_{t}")
            phase_mm(ps2, range(4, 8), parts)
            tj = gp.tile([128, 1024], bf16, tag=f"{lname}tj")
            nc.scalar.activation(tj[:], ps2[:, 0:1024], AF.Tanh)
            so = gp.tile([128, 1024], bf16, tag=f"{lname}so")
            nc.scalar.activation(so[:], ps2[:, 1024:2048], AF.Sigmoid)

            # DVE: c update (c = c*sf + si*tj), overlapped with ACT above
            si = sif[:, 0:1024]
            sf = sif[:, 1024:2048]
            if t == 0:
                nc.vector.tensor_tensor(c[:], si, tj[:], op=OP.mult)
            else:
                nc.vector.tensor_tensor(c[:], c[:], sf, op=OP.mult)
                tmp = gp.tile([128, 1024], bf16, tag=f"{lname}tmp")
                nc.vector.tensor_tensor(tmp[:], si, tj[:], op=OP.mult)
                nc.vector.tensor_tensor(c[:], c[:], tmp[:], op=OP.add)
            th = gp.tile([128, 1024], bf16, tag=f"{lname}th")
            nc.scalar.activation(th[:], c[:], AF.Tanh)
            last2 = layer == 2 and t == T_steps - 1
            if not last2:
                hdr = hp.tile([128, 2, BL], fp8, tag=f"{lname}h")
                nc.vector.tensor_tensor(hdr[:, :, :], th[:], so[:], op=OP.mult)
                if layer == 2:
                    state["h2"] = hdr
                return hdr
            else:
                h2bf = hp.tile([128, 2 * BL], bf16, tag="h2bf")
                nc.vector.tensor_tensor(h2bf[:], th[:], so[:], op=OP.mult)
                state["h2bf"] = h2bf
                return None

        # ---- pipelined main loop: slot s runs L1(s) and L2(s-1) ----
        h1_cur = None  # h1(s) after the L1 half of slot s
        for s in range(T_steps + 1):
            h1_prev = h1_cur  # h1(s-1)
            if s < T_steps:
                if s + 1 < T_steps:
                    nxt = xp.tile([E + 1, BL], bf16, tag="xt", name=f"xt{s+1}")
                    nc.sync.dma_start(nxt[:], XT[s + 1])
                h1_cur = emit_layer(1, s, h1_prev)
                if s + 1 < T_steps:
                    state["xt"] = nxt
            if s >= 1:
                # L2(s-1) consumes h1(s-1)
                emit_layer(2, s - 1, h1_prev if s < T_steps else h1_cur)

        # ---- dense + softmax cross-entropy on the last h2 ----
        h2bf = state["h2bf"]
        pdall = pp.tile([128, 2048], f32, tag="ph", name="pdall")
        pds, nmxs, ses, pkss = [], [], [], []
        for m in range(NB):
            pd = pdall[:, 512 * m : 512 * m + V]
            ms = slice(128 * m, 128 * (m + 1))
            nc.tensor.matmul(pd, h2bf[:, ms], wd[0][:], start=True, stop=False)
            nc.tensor.matmul(pd, h2bf[:, BL + 128 * m : BL + 128 * (m + 1)],
                             wd[1][:], start=False, stop=False)
            nc.tensor.matmul(pd, ones[:, ms], bdt[:], start=False, stop=True)
            pds.append(pd)
            mx = lp.tile([128, 1], f32, tag=f"mx{m}")
            nc.vector.reduce_max(out=mx[:], in_=pd, axis=mybir.AxisListType.X)
            nmx = lp.tile([128, 1], f32, tag=f"nmx{m}")
            nc.vector.tensor_scalar_mul(nmx[:], mx[:], -1.0)
            nmxs.append(nmx)
        for m in range(NB):
            ex = lp.tile([128, V], f32, tag=f"ex{m}")
            se = lp.tile([128, 1], f32, tag=f"se{m}")
            nc.scalar.activation(ex[:], pds[m], AF.Exp, bias=nmxs[m][:],
                                 accum_out=se[:])
            ses.append(se)
        lses = []
        for m in range(NB):
            lse = lp.tile([128, 1], f32, tag=f"lse{m}")
            nc.scalar.activation(lse[:], ses[m][:], AF.Ln)
            lses.append(lse)
            pk = lp.tile([128, V], f32, tag=f"pk{m}")
            nc.vector.tensor_tensor(pk[:], pds[m], oh_tiles[m][:], op=OP.mult)
            pks = lp.tile([128, 1], f32, tag=f"pks{m}")
            nc.vector.reduce_sum(out=pks[:], in_=pk[:], axis=mybir.AxisListType.X)
            pkss.append(pks)
        for m in range(NB):
            # loss = max + lse - picked  (lse is ln(sum exp(pred - max)))
            l0 = lp.tile([128, 1], f32, tag=f"l0{m}")
            nc.vector.tensor_tensor(l0[:], lses[m][:], pkss[m][:], op=OP.subtract)
            l1_ = lp.tile([128, 1], f32, tag=f"l1{m}")
            nc.vector.tensor_tensor(l1_[:], l0[:], nmxs[m][:], op=OP.subtract)
            nc.sync.dma_start(LOSS[m, :], l1_[:, 0:1])

    nc.compile()
    return nc


# permutation (i, j, f, o) -> (i, f, j, o) on the 4H gate axis
def _perm():
    idx = np.arange(4 * H).reshape(4, H)
    return np.concatenate([idx[0], idx[2], idx[1], idx[3]])


def _dr(w):
    """[256, M] -> DoubleRow layout [128, 2, M]."""
    return np.ascontiguousarray(w.reshape(2, 128, -1).transpose(1, 0, 2))


def _prep_inputs(features, labels, emb, W1, b1, W2, b2, Wd, bd):
    """Host-side shard + layout prep. Returns (in_maps, fb_only)."""
    import ml_dtypes

    bf16 = ml_dtypes.bfloat16
    fp8 = ml_dtypes.float8_e4m3
    perm = _perm()

    features = np.asarray(features)
    labels = np.asarray(labels)
    emb = np.asarray(emb, dtype=np.float32)
    W1p = np.asarray(W1, dtype=np.float32)[:, perm]
    W2p = np.asarray(W2, dtype=np.float32)[:, perm]
    b1p = np.asarray(b1, dtype=np.float32)[perm].copy()
    b2p = np.asarray(b2, dtype=np.float32)[perm].copy()
    fb_only = not np.any(b2p)
    # forget gate now lives at columns H..2H
    b1p[H : 2 * H] += FORGET_BIAS
    b2p[H : 2 * H] += FORGET_BIAS

    W1X = np.ascontiguousarray(
        np.concatenate([W1p[:E], b1p[None, :]], axis=0).astype(bf16))
    W1H = _dr(W1p[E:]).astype(fp8)
    W2A = _dr(W2p[:H]).astype(fp8)
    W2B = _dr(W2p[H:]).astype(fp8)
    B2R = np.ascontiguousarray(b2p[None, :].astype(bf16))
    WDc = np.ascontiguousarray(np.asarray(Wd, dtype=np.float32).astype(bf16))
    BDc = np.ascontiguousarray(
        np.asarray(bd, dtype=np.float32).reshape(1, V).astype(bf16))

    x = emb[features]  # [B, T, E] f32
    eye = np.eye(V, dtype=np.float32)

    in_maps = []
    for c in range(NCORES):
        sl = slice(c * BL, (c + 1) * BL)
        xc = x[sl].transpose(1, 2, 0)  # [T, E, BL]
        xo = np.concatenate(
            [xc, np.ones((T, 1, BL), np.float32)], axis=1)  # [T, E+1, BL]
        oh = eye[labels[sl]]
        in_maps.append({
            "XT": np.ascontiguousarray(xo.astype(bf16)),
            "OH": np.ascontiguousarray(oh),
            "W1X": W1X, "W1H": W1H, "W2A": W2A, "W2B": W2B, "B2R": B2R,
            "WD": WDc, "BD": BDc,
        })
    return in_maps, fb_only


def _get_nc(fb_only):
    key = ("nc", fb_only)
    if key not in _CACHE:
        _CACHE[key] = _build_nc(fb_only)
    return _CACHE[key]


def _run(inputs, trace=False, **spmd_kwargs):
    from concourse.bass_utils import run_bass_kernel_spmd

    in_maps, fb_only = _prep_inputs(**inputs)
    nc = _get_nc(fb_only)
    res = run_bass_kernel_spmd(
        nc, in_maps, list(range(NCORES)), trace=trace, **spmd_kwargs
    )
    rows = np.concatenate(
        [np.asarray(r["LOSS"], np.float64).ravel() for r in res.results])
    loss = np.asarray(rows.mean(), dtype=np.float32)
    return loss, res


def kernel(**inputs):
    loss, _ = _run(inputs, trace=False)
    return loss
